# revision 26
# baseline (speedup 1.0000x reference)
"""CapsuleLayer dynamic-routing kernel for 8 trn2 NeuronCores.

Problem: B=128, U=8, C=2048, J=32, S=16, 3 routing iterations.
  u_hat[b,c,j,s] = sum_u W[c,j,s,u] x[b,u,c]          (never materialized: 536MB)
  iter: c=softmax(b over C); s=sum_c c*u_hat; v=squash(s); b+=mean_b(u_hat.v)

Sharding: input capsules C split 8 ways (256/core). Per iteration each core
computes s-partials over its C-slice as matmuls (contraction (u,c_loc)=2048
against a c-scaled W), one AllReduce combines s-partials + softmax
denominators, then squash/b-update are local:
  s_un[b,(j,s)]  = sum_{u,c_loc} x[(u,c),b] * (e[c,j]*W[(u,c),(j,s)])
  A[c,(j,s)]     = sum_b x[b,(u,c)] * v[b,(j,s)]     (per-u matmuls)
  b[c,j]        += (1/B) sum_{u,s} W * A              (fused affine_mul_reduce)
Matmuls run in float32r (TF32-class, 4x faster than fp32 on trn2 PE).
All cores end with the full (identical) v, so core 0's output is the answer.

Measured (8x trn2, axon): ~138us per routing pass (3 iterations incl. 3
AllReduces) + ~20us one-time weight load; rel err vs fp32 reference ~1.5e-4.
Timeline-sim engine budget: DVE 45us (AMR b-update + W-scaling), ACT 26us
(PSUM drains, exp/sqrt), PE 25us (s/A matmuls), collectives ~45us.
"""

import numpy as np

B, U, C, J, S = 128, 8, 2048, 32, 16
N_CORES = 8
C_LOC = C // N_CORES          # 256
NCR = C_LOC // 128            # 2 partition-ranges per core
JS = J * S                    # 512
N_ITER = 3

_cache = {}


def _build(use_ar=True, reps=1, mmdt="f32r"):
    import concourse.bacc as bacc
    import concourse.mybir as mybir
    import concourse.tile as tile
    from concourse.tile import add_dep_helper

    f32 = mybir.dt.float32
    f32r = mybir.dt.float32r if mmdt == "f32r" else mybir.dt.bfloat16
    bf16 = mybir.dt.bfloat16
    AT = mybir.AluOpType
    ACT = mybir.ActivationFunctionType

    nc = bacc.Bacc("TRN2", target_bir_lowering=False, debug=False,
                   num_devices=N_CORES)

    # per-core inputs (host pre-sharded/transposed)
    xs_d = nc.dram_tensor("xs", [128, U, NCR, B], f32r, kind="ExternalInput")
    xa_d = nc.dram_tensor("xa", [B, U, NCR, 128], f32r, kind="ExternalInput")
    wa_d = nc.dram_tensor("wa", [128, U, NCR, J, S], f32r, kind="ExternalInput")

    v_d = nc.dram_tensor("v", [B, JS], f32, kind="ExternalOutput")

    AR_N1 = B * JS                 # iter-1 payload: s partials only
    AR_N = B * JS + 2 * J          # iters 2-3: s partials + D partials [1,64]

    with tile.TileContext(nc) as tc:
        with (
            tc.tile_pool(name="big", bufs=1) as big,
            tc.tile_pool(name="sm", bufs=2) as sm,
            tc.tile_pool(name="scr", bufs=2) as scr,
            tc.tile_pool(name="ps_s", bufs=1, space="PSUM") as ps_s,
            tc.tile_pool(name="ps_a", bufs=2, space="PSUM") as ps_a,
            tc.tile_pool(name="ps_t", bufs=1, space="PSUM") as ps_t,
            tc.tile_pool(name="dram", bufs=1, space="DRAM") as dram,
        ):
            # ---- resident tensors ----
            xs = big.tile([128, U, NCR, B], f32r, tag="xs")
            xa = big.tile([B, U, NCR, 128], f32r, tag="xa")
            wa = big.tile([128, U, NCR, J, S], f32r, tag="wa")
            ww = big.tile([128, U, NCR, J, S], f32r, tag="ww")
            a_sb = big.tile([128, U, NCR, J, S], f32, tag="a_sb")

            # xs first, then wa per-u chunks alternating queues so iter-1
            # s-matmuls start early; xa (A-step) last
            nc.scalar.dma_start(xs[:], xs_d[:])
            for u in range(U):
                eng = nc.sync if u % 2 == 0 else nc.scalar
                eng.dma_start(wa[:, u], wa_d[:, u])
            nc.sync.dma_start(xa[:], xa_d[:])

            b_cr = [sm.tile([128, J], f32, tag=f"b{cr}", name=f"b{cr}") for cr in range(NCR)]
            binc_cr = [sm.tile([128, J], f32, tag=f"binc{cr}", name=f"binc{cr}") for cr in range(NCR)]
            ones = sm.tile([128, 1], f32, tag="ones")
            onesr = sm.tile([1, 128], f32, tag="onesr")
            nc.vector.memset(ones[:], 1.0)
            nc.vector.memset(onesr[:], 1.0)

            for rep in range(reps):
             for it in range(N_ITER):
                first = it == 0
                last = it == N_ITER - 1

                # ---- c-weights: e = exp(b) per cr; fold into W.
                # cr=0's WW runs on gpsimd so it overlaps cr=1's AMR (both
                # feed this iteration's s-matmuls; AMR holds the DVE).
                e_cr = []
                if not first:
                    for cr in range(NCR):
                        e_t = sm.tile([128, J], f32, tag=f"e{cr}")
                        nc.scalar.activation(e_t[:], b_cr[cr][:], ACT.Exp)
                        e_cr.append(e_t)
                        e_bc = e_t[:].unsqueeze(-1).broadcast_to([128, J, S])
                        for u in range(U):
                            # cr0 fully on gpsimd (DVE is running cr1's AMR);
                            # cr1 fully on DVE (pool still busy with cr0)
                            eng = nc.gpsimd if cr == 0 else nc.vector
                            eng.tensor_tensor(
                                out=ww[:, u, cr],
                                in0=wa[:, u, cr],
                                in1=e_bc,
                                op=AT.mult,
                            )
                    # D partials over local c (partition sum): [1, NCR*J]
                    dpart_ps = ps_t.tile([1, NCR * J], f32, tag="tiny")
                    for cr in range(NCR):
                        nc.tensor.matmul(dpart_ps[:, cr * J:(cr + 1) * J],
                                         ones[:], e_cr[cr][:],
                                         start=True, stop=True)
                    dpart = sm.tile([1, NCR * J], f32, tag="dpart")
                    nc.scalar.copy(dpart[:], dpart_ps[:])

                # ---- s partials: accumulate 16 chunk matmuls into PSUM ----
                s_ps = ps_s.tile([B, JS], f32, tag="sps")
                rhs = wa if first else ww
                lhs = xs
                k = 0
                for cr in range(NCR):
                    for u in range(U):
                        nc.tensor.matmul(
                            s_ps[:],
                            lhs[:, u, cr],
                            rhs[:, u, cr].rearrange("p a b -> p (a b)"),
                            start=(k == 0), stop=(k == U * NCR - 1),
                        )
                        k += 1

                # drain (iter1: scale by 1/C since c is uniform)
                s_un = sm.tile([B, JS], f32, tag="sun")
                if first:
                    nc.vector.tensor_scalar_mul(s_un[:], s_ps[:], 1.0 / C)
                else:
                    nc.vector.tensor_copy(s_un[:], s_ps[:])

                # ---- AllReduce: s partials (+ D partials) ----
                n_ar = AR_N1 if first else AR_N
                ar_in = dram.tile([1, n_ar], f32, tag=f"ar_in{it}")
                ar_out = dram.tile([1, n_ar], f32, tag=f"ar_out{it}",
                                   addr_space="Shared")
                nc.sync.dma_start(ar_in[0, 0:B * JS], s_un[:])
                if not first:
                    nc.sync.dma_start(ar_in[0, B * JS:], dpart[:])
                if use_ar:
                    nc.gpsimd.collective_compute(
                        "AllReduce", AT.add,
                        replica_groups=[list(range(N_CORES))],
                        ins=[ar_in[:].opt()], outs=[ar_out[:].opt()],
                    )
                else:
                    nc.sync.dma_start(ar_out[:], ar_in[:])
                s_sum = sm.tile([B, JS], f32, tag="ssum")
                nc.sync.dma_start(s_sum[:], ar_out[0, 0:B * JS])

                if first:
                    s_t = s_sum
                else:
                    dsum = sm.tile([1, NCR * J], f32, tag="dsum")
                    nc.sync.dma_start(dsum[:], ar_out[0, B * JS:])
                    # fold cr halves, reciprocal, broadcast to [128, J]
                    dfold = sm.tile([1, J], f32, tag="dfold")
                    nc.vector.tensor_add(dfold[:], dsum[:, 0:J], dsum[:, J:2 * J])
                    drec = sm.tile([1, J], f32, tag="drec")
                    nc.vector.reciprocal(drec[:], dfold[:])
                    drec_ps = ps_t.tile([128, J], f32, tag="tiny")
                    nc.tensor.matmul(drec_ps[:], onesr[:], drec[:],
                                     start=True, stop=True)
                    drecb = sm.tile([128, J], f32, tag="drecb")
                    nc.vector.tensor_copy(drecb[:], drec_ps[:])
                    # s = s_sum * (1/D[j])
                    s_t = sm.tile([B, JS], f32, tag="st")
                    nc.vector.tensor_tensor(
                        out=s_t[:].rearrange("p (a b) -> p a b", b=S),
                        in0=s_sum[:].rearrange("p (a b) -> p a b", b=S),
                        in1=drecb[:].unsqueeze(-1).broadcast_to([B, J, S]),
                        op=AT.mult,
                    )

                # ---- squash (norm over J axis!) ----
                sq = sm.tile([B, JS], f32, tag="sq")
                nc.vector.tensor_mul(sq[:], s_t[:], s_t[:])
                msq = sm.tile([B, S], f32, tag="msq")
                nc.vector.tensor_reduce(
                    msq[:], sq[:].rearrange("p (a b) -> p b a", b=S),
                    axis=mybir.AxisListType.X, op=AT.add)
                rsq = sm.tile([B, S], f32, tag="rsq")
                nc.scalar.sqrt(rsq[:], msq[:])
                den = sm.tile([B, S], f32, tag="den")
                nc.vector.tensor_scalar_add(den[:], msq[:], 1.0)
                rec = sm.tile([B, S], f32, tag="rec")
                nc.vector.reciprocal(rec[:], den[:])
                fmul = sm.tile([B, S], f32, tag="fmul")
                nc.vector.tensor_mul(fmul[:], rsq[:], rec[:])
                v_t = sm.tile([B, JS], f32 if last else f32r, tag=f"vt{int(last)}")
                nc.vector.tensor_tensor(
                    out=v_t[:].rearrange("p (a b) -> p a b", b=S),
                    in0=s_t[:].rearrange("p (a b) -> p a b", b=S),
                    in1=fmul[:].unsqueeze(1).broadcast_to([B, J, S]),
                    op=AT.mult,
                )

                if last:
                    nc.sync.dma_start(v_d[:], v_t[:])
                    break

                # ---- b update: A = x^T v per (u,cr); b += (1/B) sum W*A ----
                for cr in range(NCR):
                    for u2 in range(U // 2):
                        a_ps = ps_a.tile([128, 2, JS], f32, tag="aps")
                        for h in range(2):
                            nc.tensor.matmul(a_ps[:, h], xa[:, 2 * u2 + h, cr],
                                             v_t[:], start=True, stop=True)
                        nc.scalar.copy(
                            a_sb[:, 2 * u2:2 * u2 + 2, cr].rearrange(
                                "p a b c -> p a (b c)"),
                            a_ps[:])
                    for j in range(J):
                        amr_out = scr.tile([128, U, S], f32, tag="amr_out")
                        nc.vector.affine_mul_reduce(
                            out=amr_out[:],
                            accum_out=binc_cr[cr][:, j:j + 1],
                            in0=wa[:, :, cr, j, :],
                            in1=a_sb[:, :, cr, j, :],
                            scale=1.0 / B,
                            bias=0.0,
                        )
                    if first:
                        nc.vector.tensor_copy(b_cr[cr][:], binc_cr[cr][:])
                    else:
                        nc.vector.tensor_add(b_cr[cr][:], b_cr[cr][:],
                                             binc_cr[cr][:])

    nc.compile()
    return nc


def _shard_inputs(x, W, mmdt="f32r"):
    if mmdt == "bf16":
        import ml_dtypes
        cast = lambda a: np.ascontiguousarray(a, dtype=ml_dtypes.bfloat16)
    else:
        cast = lambda a: np.ascontiguousarray(a, dtype=np.float32)
    x = np.ascontiguousarray(x, dtype=np.float32)
    W = np.ascontiguousarray(W, dtype=np.float32)
    in_maps = []
    for m in range(N_CORES):
        xc = x[:, :, m * C_LOC:(m + 1) * C_LOC]          # [B, U, 256]
        xr = xc.reshape(B, U, NCR, 128)                  # c_loc -> (cr, p)
        xs = cast(xr.transpose(3, 1, 2, 0))              # [128,U,NCR,B]
        xa = cast(xr)                                    # [B,U,NCR,128]
        Wc = W[0, m * C_LOC:(m + 1) * C_LOC]             # [256, J, S, U]
        wr = Wc.reshape(NCR, 128, J, S, U)
        wa = cast(wr.transpose(1, 4, 0, 2, 3))           # [128,U,NCR,J,S]
        in_maps.append({"xs": xs, "xa": xa, "wa": wa})
    return in_maps


MMDT = "f32r"


def run(x, W, trace=False):
    from concourse import bass_utils

    if "nc" not in _cache:
        _cache["nc"] = _build(mmdt=MMDT)
    nc = _cache["nc"]
    in_maps = _shard_inputs(x, W, mmdt=MMDT)
    res = bass_utils.run_bass_kernel_spmd(
        nc, in_maps, core_ids=list(range(N_CORES)), trace=trace)
    v = res.results[0]["v"].reshape(B, J, S, 1).astype(np.float32)
    return v, res


def kernel(x, W):
    v, _ = run(x, W)
    return v


# revision 29
# speedup vs baseline: 1.4748x; 1.4748x over previous
"""CapsuleLayer dynamic-routing kernel for 8 trn2 NeuronCores.

Problem: B=128, U=8, C=2048, J=32, S=16, 3 routing iterations.
  u_hat[b,c,j,s] = sum_u W[c,j,s,u] x[b,u,c]          (never materialized: 536MB)
  iter: c=softmax(b over C); s=sum_c c*u_hat; v=squash(s); b+=mean_b(u_hat.v)

Sharding: input capsules C split 8 ways (256/core). Per iteration each core
computes s-partials over its C-slice as matmuls (contraction (u,c_loc)=2048
against a c-scaled W), one AllReduce combines s-partials + softmax
denominators, then squash/b-update are local:
  s_un[b,(j,s)]  = sum_{u,c_loc} x[(u,c),b] * (e[c,j]*W[(u,c),(j,s)])
  A[c,(j,s)]     = sum_b x[b,(u,c)] * v[b,(j,s)]     (per-u matmuls)
  b[c,j]        += (1/B) sum_{u,s} W * A              (fused affine_mul_reduce)
Matmuls run in float32r (TF32-class, 4x faster than fp32 on trn2 PE).
All cores end with the full (identical) v, so core 0's output is the answer.

Measured (8x trn2, axon): ~138us per routing pass (3 iterations incl. 3
AllReduces) + ~20us one-time weight load; rel err vs fp32 reference ~1.5e-4.
Timeline-sim engine budget: DVE 45us (AMR b-update + W-scaling), ACT 26us
(PSUM drains, exp/sqrt), PE 25us (s/A matmuls), collectives ~45us.
"""

import numpy as np

B, U, C, J, S = 128, 8, 2048, 32, 16
N_CORES = 8
C_LOC = C // N_CORES          # 256
NCR = C_LOC // 128            # 2 partition-ranges per core
JS = J * S                    # 512
N_ITER = 3

_cache = {}


def _build(use_ar=True, reps=1, mmdt="f32r"):
    import concourse.bacc as bacc
    import concourse.mybir as mybir
    import concourse.tile as tile
    from concourse.tile import add_dep_helper

    f32 = mybir.dt.float32
    f32r = mybir.dt.float32r if mmdt == "f32r" else mybir.dt.bfloat16
    f16 = mybir.dt.float16
    AT = mybir.AluOpType
    ACT = mybir.ActivationFunctionType

    nc = bacc.Bacc("TRN2", target_bir_lowering=False, debug=False,
                   num_devices=N_CORES)

    # per-core inputs (host pre-sharded/transposed)
    xs_d = nc.dram_tensor("xs", [128, U, NCR, B], f32r, kind="ExternalInput")
    xa_d = nc.dram_tensor("xa", [B, U, NCR, 128], f32r, kind="ExternalInput")
    wa_d = nc.dram_tensor("wa", [128, U, NCR, J, S], f32r, kind="ExternalInput")

    v_d = nc.dram_tensor("v", [B, JS], f32, kind="ExternalOutput")

    AR_N1 = B * JS                 # iter-1 payload: s partials only
    AR_N = B * JS + 2 * J          # iters 2-3: s partials + D partials [1,64]

    with tile.TileContext(nc) as tc:
        with (
            tc.tile_pool(name="big", bufs=1) as big,
            tc.tile_pool(name="sm", bufs=2) as sm,
            tc.tile_pool(name="scr", bufs=2) as scr,
            tc.tile_pool(name="ps_s", bufs=1, space="PSUM") as ps_s,
            tc.tile_pool(name="ps_a", bufs=2, space="PSUM") as ps_a,
            tc.tile_pool(name="ps_t", bufs=1, space="PSUM") as ps_t,
            tc.tile_pool(name="dram", bufs=1, space="DRAM") as dram,
        ):
            # ---- resident tensors ----
            xs = big.tile([128, U, NCR, B], f32r, tag="xs")
            xa = big.tile([B, U, NCR, 128], f32r, tag="xa")
            wa = big.tile([128, U, NCR, J, S], f32r, tag="wa")
            ww = big.tile([128, U, NCR, J, S], f32r, tag="ww")
            a_sb = big.tile([128, U, NCR, J, S], f32, tag="a_sb")

            # xs first, then wa per-u chunks alternating queues so iter-1
            # s-matmuls start early; xa (A-step) last
            nc.scalar.dma_start(xs[:], xs_d[:])
            for u in range(U):
                eng = nc.sync if u % 2 == 0 else nc.scalar
                eng.dma_start(wa[:, u], wa_d[:, u])
            nc.sync.dma_start(xa[:], xa_d[:])

            b_cr = [sm.tile([128, J], f32, tag=f"b{cr}", name=f"b{cr}") for cr in range(NCR)]
            binc_cr = [sm.tile([128, J], f32, tag=f"binc{cr}", name=f"binc{cr}") for cr in range(NCR)]
            ones = sm.tile([128, 1], f32, tag="ones")
            onesr = sm.tile([1, 128], f32, tag="onesr")
            nc.vector.memset(ones[:], 1.0)
            nc.vector.memset(onesr[:], 1.0)

            for rep in range(reps):
             for it in range(N_ITER):
                first = it == 0
                last = it == N_ITER - 1
                # AllReduce payload dtype: fp16 iters 1-2, fp32 last iter
                pdt = f32 if last else f16

                # ---- c-weights: e = exp(b) per cr; fold into W.
                # cr=0's WW runs on gpsimd so it overlaps cr=1's AMR (both
                # feed this iteration's s-matmuls; AMR holds the DVE).
                e_cr = []
                if not first:
                    for cr in range(NCR):
                        e_t = sm.tile([128, J], f32, tag=f"e{cr}")
                        nc.scalar.activation(e_t[:], b_cr[cr][:], ACT.Exp)
                        e_cr.append(e_t)
                        e_bc = e_t[:].unsqueeze(-1).broadcast_to([128, J, S])
                        for u in range(U):
                            # cr0 fully on gpsimd (DVE is running cr1's AMR);
                            # cr1 fully on DVE (pool still busy with cr0)
                            eng = nc.gpsimd if cr == 0 else nc.vector
                            eng.tensor_tensor(
                                out=ww[:, u, cr],
                                in0=wa[:, u, cr],
                                in1=e_bc,
                                op=AT.mult,
                            )
                    # D partials over local c (partition sum): [1, NCR*J]
                    dpart_ps = ps_t.tile([1, NCR * J], f32, tag="tiny")
                    for cr in range(NCR):
                        nc.tensor.matmul(dpart_ps[:, cr * J:(cr + 1) * J],
                                         ones[:], e_cr[cr][:],
                                         start=True, stop=True)
                    dpart = sm.tile([1, NCR * J], pdt, tag="dpart")
                    nc.scalar.mul(dpart[:], dpart_ps[:], 1.0 / 64.0)

                # ---- s partials: accumulate 16 chunk matmuls into PSUM ----
                s_ps = ps_s.tile([B, JS], f32, tag="sps")
                rhs = wa if first else ww
                lhs = xs
                k = 0
                for cr in range(NCR):
                    for u in range(U):
                        nc.tensor.matmul(
                            s_ps[:],
                            lhs[:, u, cr],
                            rhs[:, u, cr].rearrange("p a b -> p (a b)"),
                            start=(k == 0), stop=(k == U * NCR - 1),
                        )
                        k += 1

                # drain (iter1: scale by 1/C since c is uniform)
                # iters 1-2 drain in fp16 (halves AllReduce wire + bounce
                # bytes; randn-scale partials sit well inside fp16 range).
                # The last iteration stays fp32 - its quantization would hit
                # the output directly instead of being damped by routing.
                s_un = sm.tile([B, JS], pdt, tag="sun")
                if first:
                    nc.vector.tensor_scalar_mul(s_un[:], s_ps[:], 1.0 / C)
                else:
                    nc.vector.tensor_copy(s_un[:], s_ps[:])

                # ---- AllReduce: s partials (+ D partials) ----
                n_ar = AR_N1 if first else AR_N
                ar_in = dram.tile([1, n_ar], pdt, tag=f"ar_in{it}")
                ar_out = dram.tile([1, n_ar], pdt, tag=f"ar_out{it}",
                                   addr_space="Shared")
                nc.sync.dma_start(ar_in[0, 0:B * JS], s_un[:])
                if not first:
                    nc.sync.dma_start(ar_in[0, B * JS:], dpart[:])
                if use_ar:
                    nc.gpsimd.collective_compute(
                        "AllReduce", AT.add,
                        replica_groups=[list(range(N_CORES))],
                        ins=[ar_in[:].opt()], outs=[ar_out[:].opt()],
                    )
                else:
                    nc.sync.dma_start(ar_out[:], ar_in[:])
                s_sum = sm.tile([B, JS], pdt, tag="ssum")
                nc.sync.dma_start(s_sum[:], ar_out[0, 0:B * JS])

                if first:
                    s_t = s_sum
                else:
                    dsum = sm.tile([1, NCR * J], pdt, tag="dsum")
                    nc.sync.dma_start(dsum[:], ar_out[0, B * JS:])
                    # fold cr halves, reciprocal, broadcast to [128, J]
                    dfold = sm.tile([1, J], f32, tag="dfold")
                    nc.vector.tensor_add(dfold[:], dsum[:, 0:J], dsum[:, J:2 * J])
                    drec = sm.tile([1, J], f32, tag="drec")
                    nc.vector.reciprocal(drec[:], dfold[:])
                    drec_ps = ps_t.tile([128, J], f32, tag="tiny")
                    nc.tensor.matmul(drec_ps[:], onesr[:], drec[:],
                                     start=True, stop=True)
                    drecb = sm.tile([128, J], f32, tag="drecb")
                    nc.vector.tensor_copy(drecb[:], drec_ps[:])
                    # s = s_sum * (1/D[j]); drec carries a 64x factor from
                    # the fp16 D pre-scale, compensated by the 1/64 here
                    s_t = sm.tile([B, JS], f32, tag="st")
                    nc.vector.scalar_tensor_tensor(
                        out=s_t[:].rearrange("p (a b) -> p a b", b=S),
                        in0=s_sum[:].rearrange("p (a b) -> p a b", b=S),
                        scalar=1.0 / 64.0,
                        in1=drecb[:].unsqueeze(-1).broadcast_to([B, J, S]),
                        op0=AT.mult,
                        op1=AT.mult,
                    )

                # ---- squash (norm over J axis!) ----
                sq = sm.tile([B, JS], f32, tag="sq")
                nc.vector.tensor_mul(sq[:], s_t[:], s_t[:])
                msq = sm.tile([B, S], f32, tag="msq")
                nc.vector.tensor_reduce(
                    msq[:], sq[:].rearrange("p (a b) -> p b a", b=S),
                    axis=mybir.AxisListType.X, op=AT.add)
                rsq = sm.tile([B, S], f32, tag="rsq")
                nc.scalar.sqrt(rsq[:], msq[:])
                den = sm.tile([B, S], f32, tag="den")
                nc.vector.tensor_scalar_add(den[:], msq[:], 1.0)
                rec = sm.tile([B, S], f32, tag="rec")
                nc.vector.reciprocal(rec[:], den[:])
                fmul = sm.tile([B, S], f32, tag="fmul")
                nc.vector.tensor_mul(fmul[:], rsq[:], rec[:])
                v_t = sm.tile([B, JS], f32 if last else f32r, tag=f"vt{int(last)}")
                nc.vector.tensor_tensor(
                    out=v_t[:].rearrange("p (a b) -> p a b", b=S),
                    in0=s_t[:].rearrange("p (a b) -> p a b", b=S),
                    in1=fmul[:].unsqueeze(1).broadcast_to([B, J, S]),
                    op=AT.mult,
                )

                if last:
                    nc.sync.dma_start(v_d[:], v_t[:])
                    break

                # ---- b update: A = x^T v per (u,cr); b += (1/B) sum W*A ----
                for cr in range(NCR):
                    for u2 in range(U // 2):
                        a_ps = ps_a.tile([128, 2, JS], f32, tag="aps")
                        for h in range(2):
                            nc.tensor.matmul(a_ps[:, h], xa[:, 2 * u2 + h, cr],
                                             v_t[:], start=True, stop=True)
                        nc.scalar.copy(
                            a_sb[:, 2 * u2:2 * u2 + 2, cr].rearrange(
                                "p a b c -> p a (b c)"),
                            a_ps[:])
                    for j in range(J):
                        amr_out = scr.tile([128, U, S], f32, tag="amr_out")
                        nc.vector.affine_mul_reduce(
                            out=amr_out[:],
                            accum_out=binc_cr[cr][:, j:j + 1],
                            in0=wa[:, :, cr, j, :],
                            in1=a_sb[:, :, cr, j, :],
                            scale=1.0 / B,
                            bias=0.0,
                        )
                    if first:
                        nc.vector.tensor_copy(b_cr[cr][:], binc_cr[cr][:])
                    else:
                        nc.vector.tensor_add(b_cr[cr][:], b_cr[cr][:],
                                             binc_cr[cr][:])

    nc.compile()
    return nc


def _shard_inputs(x, W, mmdt="f32r"):
    if mmdt == "bf16":
        import ml_dtypes
        cast = lambda a: np.ascontiguousarray(a, dtype=ml_dtypes.bfloat16)
    else:
        cast = lambda a: np.ascontiguousarray(a, dtype=np.float32)
    x = np.ascontiguousarray(x, dtype=np.float32)
    W = np.ascontiguousarray(W, dtype=np.float32)
    in_maps = []
    for m in range(N_CORES):
        xc = x[:, :, m * C_LOC:(m + 1) * C_LOC]          # [B, U, 256]
        xr = xc.reshape(B, U, NCR, 128)                  # c_loc -> (cr, p)
        xs = cast(xr.transpose(3, 1, 2, 0))              # [128,U,NCR,B]
        xa = cast(xr)                                    # [B,U,NCR,128]
        Wc = W[0, m * C_LOC:(m + 1) * C_LOC]             # [256, J, S, U]
        wr = Wc.reshape(NCR, 128, J, S, U)
        wa = cast(wr.transpose(1, 4, 0, 2, 3))           # [128,U,NCR,J,S]
        in_maps.append({"xs": xs, "xa": xa, "wa": wa})
    return in_maps


MMDT = "f32r"


def run(x, W, trace=False):
    from concourse import bass_utils

    if "nc" not in _cache:
        _cache["nc"] = _build(mmdt=MMDT)
    nc = _cache["nc"]
    in_maps = _shard_inputs(x, W, mmdt=MMDT)
    res = bass_utils.run_bass_kernel_spmd(
        nc, in_maps, core_ids=list(range(N_CORES)), trace=trace)
    v = res.results[0]["v"].reshape(B, J, S, 1).astype(np.float32)
    return v, res


def kernel(x, W):
    v, _ = run(x, W)
    return v
